# revision 5
# baseline (speedup 1.0000x reference)
"""Trainium2 Bass kernel for the self-attention module:

    f = conv1x1(x)            # [B, 16, N]   (w1 @ x + b1)
    E = f^T f                 # [B, N, N]    (symmetric)
    A = softmax(E, axis=-1)
    y = x + 0.1 * (x @ A^T)   # out[b,c,n] = sum_m x[b,c,m] A[b,n,m]

Sharding: 8 cores = 4 batches x 2 halves of the N=4096 rows. Each core
gets the full x[b] (column-rolled so its 2048-row half sits first) and
produces y[:, :2048] for that layout.

Device algorithm per core (all matmuls fp32r = single-pass FP22):
  - f = w1T^T @ x + b1                          [16, 4096]
  - per 512-wide n-block, per 128-wide m-chunk:
      E_psum[m,n]  = f[:,m]^T @ f[:,n]          (K=16)
      P_sbuf       = exp(E_psum)                (ScalarE; no max-sub:
                                                 E<=~54 so exp<3e23 fits
                                                 fp32, and the row max
                                                 cancels in the ratio)
      out_psum[c,n]  += xT[m,c]^T @ P           (c in 2 chunks of 128)
      cs_psum[1,n]   += tens^T @ P              (tens = 10.0-vector ->
                                                 cs = 10*colsum)
  - y = x + out * reciprocal(broadcast(cs))     (= x + 0.1*out/colsum)
"""

import numpy as np

B, C, N = 4, 256, 64 * 64
K = 16
HALF = N // 2          # rows per core
NB = HALF // 512       # 4 n-blocks of 512
MC = N // 128          # 32 m-chunks of 128
N_CORES = 8

_CACHE: dict = {}


def _emit_body(nc, sb1, sbp, sbo, sbe, ps_f, ps_e, ps_o, ps_c,
               x_d, xT_d, w1T_d, b1_d, y_d, f32, f32r, AF):
    # ---- load inputs ----
    xf0 = sb1.tile([128, N], f32r, tag="xf0")
    xf1 = sb1.tile([128, N], f32r, tag="xf1")
    nc.sync.dma_start(out=xf0, in_=x_d[0:128, :].bitcast(f32r))
    nc.sync.dma_start(out=xf1, in_=x_d[128:256, :].bitcast(f32r))
    xT = sb1.tile([128, MC, C], f32r, tag="xT")
    nc.sync.dma_start(out=xT,
                      in_=xT_d.rearrange("(j p) c -> p j c", p=128).bitcast(f32r))
    w1T = sb1.tile([128, 2, K], f32r, tag="w1T")
    nc.sync.dma_start(out=w1T,
                      in_=w1T_d.rearrange("(cc p) k -> p cc k", p=128).bitcast(f32r))
    b1 = sb1.tile([K, 1], f32, tag="b1")
    nc.sync.dma_start(out=b1, in_=b1_d)
    tens_f = sb1.tile([128, 1], f32, tag="tens_f")
    nc.vector.memset(tens_f, 10.0)
    tens = sb1.tile([128, 1], f32r, tag="tens")
    nc.vector.tensor_copy(out=tens, in_=tens_f)
    ones1_f = sb1.tile([1, 128], f32, tag="ones1_f")
    nc.vector.memset(ones1_f, 1.0)
    ones1 = sb1.tile([1, 128], f32r, tag="ones1")
    nc.vector.tensor_copy(out=ones1, in_=ones1_f)

    # ---- f = w1 @ x + b1 : [K, N] ----
    f_sb = sb1.tile([K, N], f32r, tag="f")
    for mj in range(N // 512):
        fp = ps_f.tile([K, 512], f32, tag="f")
        nc.tensor.matmul(fp, lhsT=w1T[:, 0, :],
                         rhs=xf0[:, mj * 512:(mj + 1) * 512],
                         start=True, stop=False)
        nc.tensor.matmul(fp, lhsT=w1T[:, 1, :],
                         rhs=xf1[:, mj * 512:(mj + 1) * 512],
                         start=False, stop=True)
        nc.vector.tensor_scalar_add(
            out=f_sb[:, mj * 512:(mj + 1) * 512], in0=fp, scalar1=b1)

    # ---- main: attention ----
    for j in range(NB):
        nsl = slice(j * 512, (j + 1) * 512)
        out0 = ps_o.tile([128, 512], f32, tag="c0")
        out1 = ps_o.tile([128, 512], f32, tag="c1")
        cs = ps_c.tile([1, 512], f32, tag="cs")
        for i in range(MC):
            ep = ps_e.tile([128, 512], f32, tag="e")
            nc.tensor.matmul(ep,
                             lhsT=f_sb[:, i * 128:(i + 1) * 128],
                             rhs=f_sb[:, nsl],
                             start=True, stop=True)
            p = sbp.tile([128, 512], f32r, tag="p")
            nc.scalar.activation(out=p, in_=ep, func=AF.Exp)
            nc.tensor.matmul(out0, lhsT=xT[:, i, 0:128],
                             rhs=p, start=(i == 0), stop=(i == MC - 1))
            nc.tensor.matmul(out1, lhsT=xT[:, i, 128:256],
                             rhs=p, start=(i == 0), stop=(i == MC - 1))
            nc.tensor.matmul(cs, lhsT=tens, rhs=p,
                             start=(i == 0), stop=(i == MC - 1))
        # epilogue: y[:, nsl] = x[:, nsl] + out * (0.1 / colsum)
        cs_sb = sbe.tile([1, 512], f32r, tag="cs_sb")
        nc.vector.tensor_copy(out=cs_sb, in_=cs)
        bc = ps_c.tile([128, 512], f32, tag="bc")
        nc.tensor.matmul(bc, lhsT=ones1, rhs=cs_sb, start=True, stop=True)
        rec = sbe.tile([128, 512], f32, tag="rec")
        nc.vector.reciprocal(out=rec, in_=bc)
        for cc, outp, xfc in ((0, out0, xf0), (1, out1, xf1)):
            yo = sbo.tile([128, 512], f32, tag="yo")
            nc.vector.tensor_mul(yo, outp, rec)
            nc.vector.tensor_add(yo, yo, xfc[:, nsl].bitcast(f32))
            nc.sync.dma_start(out=y_d[cc * 128:(cc + 1) * 128, nsl], in_=yo)


def _build(loop_reps=None):
    from contextlib import ExitStack

    import concourse.mybir as mybir
    import concourse.tile as tile
    from concourse import bacc

    f32 = mybir.dt.float32
    f32r = mybir.dt.float32r
    AF = mybir.ActivationFunctionType

    nc = bacc.Bacc("TRN2", target_bir_lowering=False, debug=False,
                   num_devices=N_CORES)
    x_d = nc.dram_tensor("x", [C, N], f32, kind="ExternalInput").ap()
    xT_d = nc.dram_tensor("xT", [N, C], f32, kind="ExternalInput").ap()
    w1T_d = nc.dram_tensor("w1T", [C, K], f32, kind="ExternalInput").ap()
    b1_d = nc.dram_tensor("b1", [K, 1], f32, kind="ExternalInput").ap()
    y_d = nc.dram_tensor("y", [C, HALF], f32, kind="ExternalOutput").ap()

    with tile.TileContext(nc) as tc, ExitStack() as ctx:
        sb1 = ctx.enter_context(tc.tile_pool(name="sb1", bufs=1))
        sbp = ctx.enter_context(tc.tile_pool(name="sbp", bufs=3))
        sbo = ctx.enter_context(tc.tile_pool(name="sbo", bufs=3))
        sbe = ctx.enter_context(tc.tile_pool(name="sbe", bufs=2))
        ps_f = ctx.enter_context(tc.tile_pool(name="psf", bufs=2, space="PSUM"))
        ps_e = ctx.enter_context(tc.tile_pool(name="pse", bufs=2, space="PSUM"))
        ps_o = ctx.enter_context(tc.tile_pool(name="pso", bufs=1, space="PSUM"))
        ps_c = ctx.enter_context(tc.tile_pool(name="psc", bufs=1, space="PSUM"))

        args = (nc, sb1, sbp, sbo, sbe, ps_f, ps_e, ps_o, ps_c,
                x_d, xT_d, w1T_d, b1_d, y_d, f32, f32r, AF)
        if loop_reps is None:
            _emit_body(*args)
        else:
            with tc.For_i(0, loop_reps, 1,
                          hint_engines=(mybir.EngineType.PE,
                                        mybir.EngineType.Activation,
                                        mybir.EngineType.DVE)):
                _emit_body(*args)

    nc.compile()
    return nc


def _get_nc(loop_reps=None):
    key = ("nc", loop_reps)
    if key not in _CACHE:
        _CACHE[key] = _build(loop_reps)
    return _CACHE[key]


def _make_in_maps(x, w1, b1):
    xf = np.ascontiguousarray(x.reshape(B, C, N), dtype=np.float32)
    w1T = np.ascontiguousarray(w1.T, dtype=np.float32)
    b1c = np.ascontiguousarray(b1.reshape(K, 1), dtype=np.float32)
    in_maps = []
    for core in range(N_CORES):
        b, h = divmod(core, 2)
        xs = xf[b] if h == 0 else np.roll(xf[b], -HALF, axis=1)
        in_maps.append({
            "x": np.ascontiguousarray(xs),
            "xT": np.ascontiguousarray(xs.T),
            "w1T": w1T,
            "b1": b1c,
        })
    return in_maps


def kernel(x, w1, b1):
    from concourse.bass_utils import run_bass_kernel_spmd

    nc = _get_nc()
    in_maps = _make_in_maps(x, w1, b1)
    res = run_bass_kernel_spmd(nc, in_maps, list(range(N_CORES)))
    out = np.empty((B, C, N), np.float32)
    for core in range(N_CORES):
        b, h = divmod(core, 2)
        out[b, :, h * HALF:(h + 1) * HALF] = res.results[core]["y"]
    return out.reshape(x.shape).astype(x.dtype, copy=False)
